# revision 59
# baseline (speedup 1.0000x reference)
"""BottleneckMamba Trainium2 kernel (self-contained).

out = x + cv2( scale * out_proj( LN(cross-merge(4-dir selective scan(N=1))) * z ) )

3 SPMD launches on 8 NeuronCores:
  L1 (core=(b, image-half)): cv1 -> h; depthwise3x3*in_proj folded into 9
     matmuls -> silu -> xc ; z = silu(Wz@h) ; B/C projection rows.
  L2 (core=(b, dir-group)): per direction (fwd/rev over its u layout):
     dtd matmul -> softplus (ACT) -> av=exp(A*dt) (ACT) -> bt=dt*ub (DVE,
     ub=u*B premultiplied on host) -> warmup-window scans (chunks are
     independent: state influence decays below 1e-14 within 64 cols, so
     each 2048-chunk scans [chunk-64, chunk_end) from state 0) split
     across DVE and Pool engines. Raw h written out per direction.
  L3 (core=(b, half)): y centered via (I-J/128) matmul, var via (J/128)
     matmul of y~^2, rstd on ACT, t=(y~*rstd)*z, final fused
     (cv2 @ diag(scale) @ out_proj @ diag(ln_g)) matmul -> delta bf16.
Host: shards/reassembles, transposes, premultiplies u*B, merges
  h_f*C_f + h_r*C_r + D*xc pairs, adds residual x + cv2 bias.
"""
import os
import sys

sys.path.insert(0, '/opt/trn_rl_repo')

import numpy as np
import ml_dtypes

import concourse.bass as bass
import concourse.tile as tile
import concourse.mybir as mybir
from concourse.bass_utils import run_bass_kernel_spmd

bf16 = mybir.dt.bfloat16
fp8 = mybir.dt.float8e4
f32 = mybir.dt.float32
NF8 = ml_dtypes.float8_e4m3
DR = mybir.MatmulPerfMode.DoubleRow
FS = 32.0  # fp8 fold-weight prescale (values ~0.01 would be subnormal in e4m3)
MULT, ADD = mybir.AluOpType.mult, mybir.AluOpType.add
SUB = mybir.AluOpType.subtract
AF = mybir.ActivationFunctionType
NBF = ml_dtypes.bfloat16

B, C1, C2, H, W = 4, 256, 256, 128, 128
Cm, K, R = 128, 4, 8
L = H * W          # 16384
HH = H // 2        # 64 rows per half
LH = HH * W        # 8192
CH = 2048          # L2 chunk
NCH = L // CH      # 8
WU = 64            # scan warmup columns
LP = L + 2 * WU    # padded length 16512

EXEC_TIMES = {}    # launch -> exec ns (MAMBA_TRACE=1)
_CACHE = {}


def _split_multiwaits(nc):
    """walrus here accepts ONE sync-wait per instruction; hoist extras into
    single-wait same-engine NOPs inserted before the instruction."""
    for f in nc.m.functions:
        for bb in f.blocks:
            il = bb.instructions
            i = 0
            while i < len(il):
                ins = il[i]
                si = getattr(ins, "sync_info", None)
                if si is not None and len(si.on_wait) > 1:
                    waits = list(si.on_wait)
                    ins.sync_info = mybir.SyncInfo(
                        on_wait=[waits[-1]], on_update=list(si.on_update))
                    for w in waits[:-1]:
                        nop = mybir.InstNoOp(
                            name=nc.get_next_instruction_name(), ins=[], outs=[])
                        nop.engine = ins.engine
                        nop.sync_info = mybir.SyncInfo(on_wait=[w], on_update=[])
                        nc.register_instruction(nop, overwrite=True)
                        il.insert(i, nop)
                        i += 1
                i += 1


def _new_nc():
    return bass.Bass("TRN2", target_bir_lowering=False, debug=False,
                     enable_asserts=True, num_devices=8)


def _run(nc, in_maps, name):
    trace = os.environ.get("MAMBA_TRACE", "0") == "1"
    res = run_bass_kernel_spmd(nc, in_maps, core_ids=list(range(8)), trace=trace)
    if trace:
        EXEC_TIMES[name] = res.exec_time_ns
    return res.results


# ------------------------------------------------------------------- L1
def build_l1():
    nc = _new_nc()
    x_in = nc.dram_tensor("x_in", [C1, HH + 2, W], bf16, kind="ExternalInput")
    wcv1 = nc.dram_tensor("wcv1", [C1, Cm], bf16, kind="ExternalInput")       # lhsT
    bcv1 = nc.dram_tensor("bcv1", [Cm, 1], f32, kind="ExternalInput")
    wfold = nc.dram_tensor("wfold", [Cm, 9, Cm], bf16, kind="ExternalInput")  # (k, tap, m)
    bconv = nc.dram_tensor("bconv", [Cm, 1], f32, kind="ExternalInput")
    wz = nc.dram_tensor("wz", [Cm, Cm], bf16, kind="ExternalInput")           # lhsT
    wbc = nc.dram_tensor("wbc", [Cm, 8], bf16, kind="ExternalInput")          # lhsT
    hmask = nc.dram_tensor("hmask", [Cm, 2], f32, kind="ExternalInput")
    xc_out = nc.dram_tensor("xc_out", [Cm, LH], bf16, kind="ExternalOutput")
    z_out = nc.dram_tensor("z_out", [Cm, LH], bf16, kind="ExternalOutput")
    bcr_out = nc.dram_tensor("bcr_out", [8, LH], bf16, kind="ExternalOutput")

    HP = HH + 2   # 66
    WP = W + 2    # 130

    with tile.TileContext(nc) as tc, \
         tc.tile_pool(name="w", bufs=1) as wp, \
         tc.tile_pool(name="d", bufs=1) as dp, \
         tc.tile_pool(name="ps", bufs=2, space="PSUM") as pp, \
         tc.tile_pool(name="ps8", bufs=2, space="PSUM") as pp8:
        tw1a = wp.tile([128, Cm], bf16)
        tw1b = wp.tile([128, Cm], bf16)
        nc.sync.dma_start(out=tw1a, in_=wcv1[0:128, :])
        nc.sync.dma_start(out=tw1b, in_=wcv1[128:256, :])
        twf = wp.tile([Cm, 9, Cm], bf16)
        nc.sync.dma_start(out=twf, in_=wfold[:, :, :])
        twz = wp.tile([Cm, Cm], bf16)
        nc.sync.dma_start(out=twz, in_=wz[:, :])
        twbc = wp.tile([Cm, 8], bf16)
        nc.sync.dma_start(out=twbc, in_=wbc[:, :])
        tb1 = wp.tile([Cm, 1], f32)
        nc.sync.dma_start(out=tb1, in_=bcv1[:, :])
        tbc = wp.tile([Cm, 1], f32)
        nc.sync.dma_start(out=tbc, in_=bconv[:, :])
        tmask = wp.tile([Cm, 2], f32)
        nc.sync.dma_start(out=tmask, in_=hmask[:, :])

        txa = dp.tile([128, HP, W], bf16)
        txb = dp.tile([128, HP, W], bf16)
        for rb in range(0, HP, 11):
            nc.sync.dma_start(out=txa[:, rb:rb + 11, :], in_=x_in[0:128, rb:rb + 11, :])
            nc.sync.dma_start(out=txb[:, rb:rb + 11, :], in_=x_in[128:256, rb:rb + 11, :])

        th = dp.tile([Cm, HP, WP], bf16)
        nc.vector.memset(th[:, :, 0:1], 0.0)
        nc.vector.memset(th[:, :, WP - 1:WP], 0.0)

        # cv1 over 66 rows: 16 chunks of 4 rows + 1 chunk of 2 rows
        row_chunks = [(r0, 4) for r0 in range(0, 64, 4)] + [(64, 2)]
        for r0, nr in row_chunks:
            pt = pp.tile([Cm, 512], f32, tag="cv1")
            nn = nr * W
            nc.tensor.matmul(out=pt[:, :nn], lhsT=tw1a[:, :],
                             rhs=txa[:, r0:r0 + nr, :], start=True, stop=False)
            nc.tensor.matmul(out=pt[:, :nn], lhsT=tw1b[:, :],
                             rhs=txb[:, r0:r0 + nr, :], start=False, stop=True)
            nc.scalar.activation(out=th[:, r0:r0 + nr, 1:W + 1], in_=pt[:, :nn],
                                 func=AF.Identity, bias=tb1[:, :], scale=1.0)
        nc.vector.tensor_scalar_mul(out=th[:, 0, :], in0=th[:, 0, :],
                                    scalar1=tmask[:, 0:1])
        nc.vector.tensor_scalar_mul(out=th[:, HP - 1, :], in0=th[:, HP - 1, :],
                                    scalar1=tmask[:, 1:2])
        txc = dp.tile([Cm, HH, W], bf16)
        tz = dp.tile([Cm, HH, W], bf16)
        tbcr = dp.tile([8, LH], bf16)
        for r0 in range(0, HH, 4):
            pt = pp.tile([Cm, 512], f32, tag="fold")
            for t in range(9):
                dy, dx = t // 3 - 1, t % 3 - 1
                nc.tensor.matmul(
                    out=pt[:, :], lhsT=twf[:, t, :],
                    rhs=th[:, r0 + 1 + dy:r0 + 5 + dy, 1 + dx:W + 1 + dx],
                    start=(t == 0), stop=(t == 8))
            nc.scalar.activation(out=txc[:, r0:r0 + 4, :], in_=pt[:, :],
                                 func=AF.Silu, bias=tbc[:, :], scale=1.0)
            ptz = pp.tile([Cm, 512], f32, tag="z")
            nc.tensor.matmul(out=ptz[:, :], lhsT=twz[:, :],
                             rhs=th[:, r0 + 1:r0 + 5, 1:W + 1],
                             start=True, stop=True)
            nc.scalar.activation(out=tz[:, r0:r0 + 4, :], in_=ptz[:, :],
                                 func=AF.Silu, bias=0.0, scale=1.0)
            ptb = pp8.tile([8, 512], f32, tag="bc")
            nc.tensor.matmul(out=ptb[:, :], lhsT=twbc[:, :],
                             rhs=txc[:, r0:r0 + 4, :], start=True, stop=True)
            nc.vector.tensor_copy(out=tbcr[:, r0 * W:(r0 + 4) * W], in_=ptb[:, :])

            if r0 % 16 == 12:  # flush every 16 rows
                rs = r0 - 12
                nc.sync.dma_start(out=xc_out[:, rs * W:(r0 + 4) * W],
                                  in_=txc[:, rs:r0 + 4, :])
                nc.sync.dma_start(out=z_out[:, rs * W:(r0 + 4) * W],
                                  in_=tz[:, rs:r0 + 4, :])
        nc.sync.dma_start(out=bcr_out[:, :], in_=tbcr[:, :])
    return nc


# ------------------------------------------------------------------- L2
# tensor_tensor_scan only runs on DVE (compiler rejects it on Pool), and
# Pool tensor_tensor computes garbage on HW, so all elementwise stays on
# DVE. Reversed-AP scans cost ~2x, so the host pre-reverses the reverse
# direction's arrays and both directions scan forward here.


def build_l2():
    nc = _new_nc()
    u_f = nc.dram_tensor("u_f", [Cm, LP], bf16, kind="ExternalInput")
    u_r = nc.dram_tensor("u_r", [Cm, LP], bf16, kind="ExternalInput")
    ub_f = nc.dram_tensor("ub_f", [Cm, LP], bf16, kind="ExternalInput")
    ub_r = nc.dram_tensor("ub_r", [Cm, LP], bf16, kind="ExternalInput")
    wdt_f = nc.dram_tensor("wdt_f", [Cm, Cm], bf16, kind="ExternalInput")
    wdt_r = nc.dram_tensor("wdt_r", [Cm, Cm], bf16, kind="ExternalInput")
    dtb_f = nc.dram_tensor("dtb_f", [Cm, 1], f32, kind="ExternalInput")
    dtb_r = nc.dram_tensor("dtb_r", [Cm, 1], f32, kind="ExternalInput")
    a_f = nc.dram_tensor("a_f", [Cm, 1], f32, kind="ExternalInput")
    a_r = nc.dram_tensor("a_r", [Cm, 1], f32, kind="ExternalInput")
    hf_out = nc.dram_tensor("hf_out", [Cm, L], bf16, kind="ExternalOutput")
    hr_out = nc.dram_tensor("hr_out", [Cm, L], bf16, kind="ExternalOutput")

    EXT = CH + WU  # 2112
    with tile.TileContext(nc) as tc, \
         tc.tile_pool(name="w", bufs=1) as wp, \
         tc.tile_pool(name="u", bufs=1) as up, \
         tc.tile_pool(name="e1", bufs=2) as e1p, \
         tc.tile_pool(name="av", bufs=2) as avp, \
         tc.tile_pool(name="bt", bufs=2) as btp, \
         tc.tile_pool(name="h", bufs=3) as hp, \
         tc.tile_pool(name="psa", bufs=2, space="PSUM") as psa, \
         tc.tile_pool(name="psb", bufs=1, space="PSUM") as psb:
        twf_ = wp.tile([Cm, Cm], bf16)
        twr_ = wp.tile([Cm, Cm], bf16)
        nc.sync.dma_start(out=twf_, in_=wdt_f[:, :])
        nc.sync.dma_start(out=twr_, in_=wdt_r[:, :])
        tbf = wp.tile([Cm, 1], f32)
        tbr = wp.tile([Cm, 1], f32)
        taf = wp.tile([Cm, 1], f32)
        tar = wp.tile([Cm, 1], f32)
        nc.sync.dma_start(out=tbf, in_=dtb_f[:, :])
        nc.sync.dma_start(out=tbr, in_=dtb_r[:, :])
        nc.sync.dma_start(out=taf, in_=a_f[:, :])
        nc.sync.dma_start(out=tar, in_=a_r[:, :])

        tuf = up.tile([Cm, LP], bf16)
        tur = up.tile([Cm, LP], bf16)
        tubf = up.tile([Cm, LP], bf16)
        tubr = up.tile([Cm, LP], bf16)
        SL = LP // 8  # 2064
        for i in range(8):
            s = slice(i * SL, (i + 1) * SL)
            nc.sync.dma_start(out=tuf[:, s], in_=u_f[:, s])
            nc.sync.dma_start(out=tur[:, s], in_=u_r[:, s])
            nc.sync.dma_start(out=tubf[:, s], in_=ub_f[:, s])
            nc.sync.dma_start(out=tubr[:, s], in_=ub_r[:, s])

        dirs = {
            'f': (twf_, tbf, taf, tuf, tubf, hf_out),
            'r': (twr_, tbr, tar, tur, tubr, hr_out),
        }
        PIECES = [(0, 1024), (1024, 1088)]

        def chunk(d, ci):
            tw, tb, ta, tu, tub, hout = dirs[d]
            sp0 = ci * CH  # padded-coords span start, len EXT
            # softplus via exp->ln (both in the natural_log_exp ACT table)
            te1 = e1p.tile([Cm, EXT], bf16, tag="e1" + d)
            for po, pl in PIECES:
                pool = psa if po == 0 else psb
                pt = pool.tile([Cm, pl], f32, tag="dtd%d" % po)
                for so in range(0, pl, 512):
                    sl_len = min(512, pl - so)
                    nc.tensor.matmul(
                        out=pt[:, so:so + sl_len], lhsT=tw[:, :],
                        rhs=tu[:, sp0 + po + so: sp0 + po + so + sl_len],
                        start=True, stop=True)
                nc.scalar.activation(out=te1[:, po:po + pl], in_=pt[:, 0:pl],
                                     func=AF.Exp, bias=tb[:, :], scale=1.0)
            # dt = ln(1 + e1) in place: te1 becomes dt
            nc.scalar.activation(out=te1, in_=te1, func=AF.Ln,
                                 bias=1.0, scale=1.0)
            tdt = te1
            tav = avp.tile([Cm, EXT], bf16, tag="av" + d)
            nc.scalar.activation(out=tav, in_=tdt, func=AF.Exp,
                                 bias=0.0, scale=ta[:, :])
            tbt = btp.tile([Cm, EXT], bf16, tag="bt" + d)
            nc.vector.tensor_tensor(out=tbt, in0=tdt,
                                    in1=tub[:, sp0:sp0 + EXT], op=MULT)
            th = hp.tile([Cm, EXT], bf16, tag="h" + d)
            nc.vector.tensor_tensor_scan(out=th, data0=tav, data1=tbt,
                                         initial=0.0, op0=MULT, op1=ADD)
            # h-out triggers wait on the scan; keep them OFF the ACT queue
            # (an ACT-issued DMA wait would head-block later ln/exp ops).
            nc.sync.dma_start(out=hout[:, ci * CH:(ci + 1) * CH], in_=th[:, WU:EXT])

        for ci in range(NCH):
            chunk('f', ci)
            chunk('r', ci)
    return nc


# ------------------------------------------------------------------- L3
def build_l3(with_lnb):
    nc = _new_nc()
    y_in = nc.dram_tensor("y_in", [Cm, LH], bf16, kind="ExternalInput")
    z_in = nc.dram_tensor("z_in", [Cm, LH], bf16, kind="ExternalInput")
    wcen = nc.dram_tensor("wcen", [Cm, Cm], bf16, kind="ExternalInput")   # I - J/128
    wvar = nc.dram_tensor("wvar", [Cm, Cm], bf16, kind="ExternalInput")   # J/128
    wfin = nc.dram_tensor("wfin", [Cm, C2], bf16, kind="ExternalInput")   # lhsT
    wfinb = nc.dram_tensor("wfinb", [Cm, C2], bf16, kind="ExternalInput") # lhsT (ln_b path)
    d_out = nc.dram_tensor("d_out", [C2, LH], bf16, kind="ExternalOutput")

    NC3 = LH // 512  # 16
    with tile.TileContext(nc) as tc, \
         tc.tile_pool(name="w", bufs=1) as wp, \
         tc.tile_pool(name="d", bufs=1) as dp, \
         tc.tile_pool(name="c", bufs=4) as cp, \
         tc.tile_pool(name="pc", bufs=3, space="PSUM") as ppc, \
         tc.tile_pool(name="pv", bufs=2, space="PSUM") as ppv, \
         tc.tile_pool(name="pd", bufs=3, space="PSUM") as ppd:
        twc = wp.tile([Cm, Cm], bf16)
        nc.sync.dma_start(out=twc, in_=wcen[:, :])
        twv = wp.tile([Cm, Cm], bf16)
        nc.sync.dma_start(out=twv, in_=wvar[:, :])
        twa = wp.tile([Cm, 128], bf16)
        twb = wp.tile([Cm, 128], bf16)
        nc.scalar.dma_start(out=twa, in_=wfin[:, 0:128])
        nc.scalar.dma_start(out=twb, in_=wfin[:, 128:256])
        if with_lnb:
            twba = wp.tile([Cm, 128], bf16)
            twbb = wp.tile([Cm, 128], bf16)
            nc.scalar.dma_start(out=twba, in_=wfinb[:, 0:128])
            nc.scalar.dma_start(out=twbb, in_=wfinb[:, 128:256])

        teps = wp.tile([Cm, 1], f32)
        nc.vector.memset(teps, 1e-5)

        ty = dp.tile([Cm, LH], bf16)
        tz = dp.tile([Cm, LH], bf16)
        for i in range(4):
            s = slice(i * LH // 4, (i + 1) * LH // 4)
            nc.sync.dma_start(out=ty[:, s], in_=y_in[:, s])
            nc.sync.dma_start(out=tz[:, s], in_=z_in[:, s])

        # software-pipelined issue: stagger stages across chunks so each
        # in-order engine queue (esp. ACT) never head-blocks on a fresh dep.
        pcs, pvs, tts = {}, {}, {}

        def stage0(k):
            s = slice(k * 512, (k + 1) * 512)
            pc = ppc.tile([Cm, 512], f32, tag="c")
            pcs[k] = pc
            nc.tensor.matmul(out=pc, lhsT=twc[:, :], rhs=ty[:, s],
                             start=True, stop=True)
            tsq = cp.tile([Cm, 512], bf16, tag="sq")
            nc.scalar.activation(out=tsq, in_=pc, func=AF.Square,
                                 bias=0.0, scale=1.0)
            pv = ppv.tile([Cm, 512], f32, tag="v")
            pvs[k] = pv
            nc.tensor.matmul(out=pv, lhsT=twv[:, :], rhs=tsq,
                             start=True, stop=True)

        def stage1(k):
            s = slice(k * 512, (k + 1) * 512)
            tln = cp.tile([Cm, 512], f32, tag="ln")
            nc.scalar.activation(out=tln, in_=pvs.pop(k), func=AF.Ln,
                                 bias=teps[:, :], scale=1.0)
            trs = cp.tile([Cm, 512], bf16, tag="rs")
            nc.scalar.activation(out=trs, in_=tln, func=AF.Exp,
                                 bias=0.0, scale=-0.5)
            trz = cp.tile([Cm, 512], bf16, tag="rz")
            nc.vector.tensor_tensor(out=trz, in0=trs, in1=tz[:, s], op=MULT)
            tt = cp.tile([Cm, 512], bf16, tag="t")
            tts[k] = tt
            nc.vector.tensor_tensor(out=tt, in0=pcs.pop(k), in1=trz, op=MULT)

        obuf = {}

        def stage2(k):
            s = slice(k * 512, (k + 1) * 512)
            tt = tts.pop(k)
            pda = ppd.tile([Cm, 512], f32, tag="d")
            nc.tensor.matmul(out=pda, lhsT=twa[:, :], rhs=tt,
                             start=True, stop=not with_lnb)
            if with_lnb:
                nc.tensor.matmul(out=pda, lhsT=twba[:, :], rhs=tz[:, s],
                                 start=False, stop=True)
            pdb = ppd.tile([Cm, 512], f32, tag="d")
            nc.tensor.matmul(out=pdb, lhsT=twb[:, :], rhs=tt,
                             start=True, stop=not with_lnb)
            if with_lnb:
                nc.tensor.matmul(out=pdb, lhsT=twbb[:, :], rhs=tz[:, s],
                                 start=False, stop=True)
            # batch output DMAs in chunk pairs to halve sync-queue triggers
            if k % 2 == 0:
                toa = cp.tile([128, 1024], bf16, tag="oa")
                tob = cp.tile([128, 1024], bf16, tag="ob")
                obuf['a'], obuf['b'] = toa, tob
            off = (k % 2) * 512
            nc.vector.tensor_copy(out=obuf['a'][:, off:off + 512], in_=pda)
            nc.vector.tensor_copy(out=obuf['b'][:, off:off + 512], in_=pdb)
            if k % 2 == 1:
                sp = slice((k - 1) * 512, (k + 1) * 512)
                nc.sync.dma_start(out=d_out[0:128, sp], in_=obuf['a'])
                nc.sync.dma_start(out=d_out[128:256, sp], in_=obuf['b'])

        for k in range(NC3 + 2):
            if k < NC3:
                stage0(k)
            if 1 <= k:
                if k - 1 < NC3:
                    stage1(k - 1)
            if k >= 2:
                stage2(k - 2)
    return nc


# ------------------------------------------------------------------- host
def _get_ncs(with_lnb):
    key = ("ncs", with_lnb)
    if key not in _CACHE:
        nc1, nc2, nc3 = build_l1(), build_l2(), build_l3(with_lnb)
        for n in (nc1, nc2, nc3):
            _split_multiwaits(n)
        _CACHE[key] = (nc1, nc2, nc3)
    return _CACHE[key]


def kernel(x, cv1_w, cv1_b, scale_w, in_proj_w, conv_w, conv_b, x_proj_w,
           dt_w, dt_b, A_logs, Ds, ln_g, ln_b, out_proj_w, cv2_w, cv2_b):
    f = np.float32
    x = np.asarray(x, f)
    cv1_w = np.asarray(cv1_w, f); cv1_b = np.asarray(cv1_b, f)
    in_proj_w = np.asarray(in_proj_w, f)
    conv_w = np.asarray(conv_w, f); conv_b = np.asarray(conv_b, f)
    x_proj_w = np.asarray(x_proj_w, f)
    dt_w = np.asarray(dt_w, f); dt_b = np.asarray(dt_b, f)
    A_logs = np.asarray(A_logs, f); Ds = np.asarray(Ds, f)
    ln_g = np.asarray(ln_g, f); ln_b = np.asarray(ln_b, f)
    out_proj_w = np.asarray(out_proj_w, f)
    cv2_w = np.asarray(cv2_w, f); cv2_b = np.asarray(cv2_b, f)
    scale_v = np.asarray(scale_w, f).reshape(Cm)

    Wip_x, Wip_z = in_proj_w[:Cm], in_proj_w[Cm:]
    dwk = conv_w[:, 0]
    A = -np.exp(A_logs).reshape(K, Cm)
    Dk = Ds.reshape(K, Cm)
    Dall = Dk.sum(axis=0)
    W_dtk = np.einsum('kdr,krc->kdc', dt_w, x_proj_w[:, :R])
    WB, WC = x_proj_w[:, R], x_proj_w[:, R + 1]
    W_final = cv2_w @ (scale_v[:, None] * out_proj_w)
    W1 = W_final * ln_g[None, :]
    with_lnb = bool(np.any(ln_b != 0.0))
    Wb1 = W_final * ln_b[None, :]

    # fold lhsT: (tap, k=h-chan, m=out-chan) -> host layout (k, tap, m)
    Wfold = np.einsum('cyx,cd->yxdc', dwk, Wip_x)      # (3,3, in, out)
    wfold_rm = np.ascontiguousarray(
        Wfold.reshape(9, Cm, Cm).transpose(1, 0, 2))   # row-major cores
    wbc_l = np.stack([WB[0], WC[0], WB[2], WC[2],
                      WB[1], WC[1], WB[3], WC[3]], axis=1)

    nc1, nc2, nc3 = _get_ncs(with_lnb)

    # ---------------- L1 ----------------
    l1_maps = []
    wcv1_h = np.ascontiguousarray(cv1_w.T).astype(NBF)
    wfold_h = wfold_rm.astype(NBF)
    wz_h = np.ascontiguousarray(Wip_z.T).astype(NBF)
    wbc_h = np.ascontiguousarray(wbc_l).astype(NBF)
    for core in range(8):
        b, half = core // 2, core % 2
        r0 = half * HH
        xs = np.zeros((C1, HH + 2, W), np.float32)
        lo, hi = r0 - 1, r0 + HH + 1
        slo, shi = max(lo, 0), min(hi, H)
        xs[:, slo - lo: shi - lo, :] = x[b, :, slo:shi, :]
        mask = np.ones((Cm, 2), np.float32)
        mask[:, 0] = 0.0 if half == 0 else 1.0
        mask[:, 1] = 1.0 if half == 0 else 0.0
        l1_maps.append({
            "x_in": xs.astype(NBF),
            "wcv1": wcv1_h,
            "bcv1": cv1_b.reshape(Cm, 1),
            "wfold": wfold_h,
            "bconv": conv_b.reshape(Cm, 1),
            "wz": wz_h,
            "wbc": wbc_h,
            "hmask": mask,
        })
    r1 = _run(nc1, l1_maps, "L1")

    xc = np.zeros((B, Cm, L), NBF)
    zf = np.zeros((B, Cm, L), NBF)
    rows = np.zeros((B, 8, L), f)
    for core in range(8):
        b, half = core // 2, core % 2
        sl = slice(half * LH, (half + 1) * LH)
        xc[b][:, sl] = r1[core]["xc_out"]
        zf[b][:, sl] = r1[core]["z_out"]
        rows[b][:, sl] = r1[core]["bcr_out"].astype(f)

    # ---------------- L2 ----------------
    def t_spatial(a):
        return np.ascontiguousarray(
            a.reshape(*a.shape[:-1], H, W).swapaxes(-1, -2).reshape(*a.shape[:-1], L))

    l2_maps = []
    crows = []  # per core (Cf, Cr) for host merge
    for core in range(8):
        b, g = core // 2, core % 2
        if g == 0:
            u = xc[b].astype(f)
            kf, kr = 0, 2
            br_f, cr_f = rows[b][0], rows[b][1]
            br_r, cr_r = rows[b][2], rows[b][3]
        else:
            u = t_spatial(xc[b]).astype(f)
            kf, kr = 1, 3
            br_f, cr_f = t_spatial(rows[b][4]), t_spatial(rows[b][5])
            br_r, cr_r = t_spatial(rows[b][6]), t_spatial(rows[b][7])
        upf = np.zeros((Cm, LP), f)
        upf[:, WU:WU + L] = u
        upr = np.zeros((Cm, LP), f)
        upr[:, WU:WU + L] = u[:, ::-1]
        ubf = np.zeros((Cm, LP), f)
        ubf[:, WU:WU + L] = u * br_f[None, :]
        ubr = np.zeros((Cm, LP), f)
        ubr[:, WU:WU + L] = (u * br_r[None, :])[:, ::-1]
        crows.append((cr_f, cr_r))
        l2_maps.append({
            "u_f": upf.astype(NBF),
            "u_r": upr.astype(NBF),
            "ub_f": ubf.astype(NBF),
            "ub_r": ubr.astype(NBF),
            "wdt_f": np.ascontiguousarray(W_dtk[kf].T).astype(NBF),
            "wdt_r": np.ascontiguousarray(W_dtk[kr].T).astype(NBF),
            "dtb_f": dt_b[kf].reshape(Cm, 1), "dtb_r": dt_b[kr].reshape(Cm, 1),
            "a_f": A[kf].reshape(Cm, 1).astype(f), "a_r": A[kr].reshape(Cm, 1).astype(f),
        })
    r2 = _run(nc2, l2_maps, "L2")

    # host merge: m_g = hf*Cf + hr*Cr ; y = m_0 + t(m_1) + Dall*xc
    l3_maps = []
    wcen_h = (np.eye(Cm, dtype=f) - np.full((Cm, Cm), 1.0 / Cm, f)).astype(NBF)
    wvar_h = np.full((Cm, Cm), 1.0 / Cm, f).astype(NBF)
    wfin_h = np.ascontiguousarray(W1.T).astype(NBF)
    wfinb_h = np.ascontiguousarray(Wb1.T).astype(NBF)
    for b in range(B):
        cf0, cr0 = crows[2 * b]
        cf1, cr1 = crows[2 * b + 1]
        m0 = (r2[2 * b]["hf_out"].astype(f) * cf0[None, :]
              + r2[2 * b]["hr_out"][:, ::-1].astype(f) * cr0[None, :])
        m1 = (r2[2 * b + 1]["hf_out"].astype(f) * cf1[None, :]
              + r2[2 * b + 1]["hr_out"][:, ::-1].astype(f) * cr1[None, :])
        ypre = (m0 + t_spatial(m1) + Dall[:, None] * xc[b].astype(f)).astype(NBF)
        for half in range(2):
            sl = slice(half * LH, (half + 1) * LH)
            l3_maps.append({
                "y_in": np.ascontiguousarray(ypre[:, sl]),
                "z_in": np.ascontiguousarray(zf[b][:, sl]),
                "wcen": wcen_h,
                "wvar": wvar_h,
                "wfin": wfin_h,
                "wfinb": wfinb_h,
            })
    r3 = _run(nc3, l3_maps, "L3")

    out = np.empty((B, C2, H, W), np.float32)
    for core in range(8):
        b, half = core // 2, core % 2
        sl = slice(half * LH, (half + 1) * LH)
        out[b].reshape(C2, L)[:, sl] = r3[core]["d_out"].astype(f)
    out += x
    out += cv2_b[None, :, None, None]
    return out


# revision 60
# speedup vs baseline: 1.1829x; 1.1829x over previous
"""BottleneckMamba Trainium2 kernel (self-contained).

out = x + cv2( scale * out_proj( LN(cross-merge(4-dir selective scan(N=1))) * z ) )

3 SPMD launches on 8 NeuronCores:
  L1 (core=(b, image-half)): cv1 -> h; depthwise3x3*in_proj folded into 9
     matmuls -> silu -> xc ; z = silu(Wz@h) ; B/C projection rows.
  L2 (core=(b, dir-group)): per direction (fwd/rev over its u layout):
     dtd matmul -> softplus (ACT) -> av=exp(A*dt) (ACT) -> bt=dt*ub (DVE,
     ub=u*B premultiplied on host) -> warmup-window scans (chunks are
     independent: state influence decays below 1e-14 within 64 cols, so
     each 2048-chunk scans [chunk-64, chunk_end) from state 0) split
     across DVE and Pool engines. Raw h written out per direction.
  L3 (core=(b, half)): y centered via (I-J/128) matmul, var via (J/128)
     matmul of y~^2, rstd on ACT, t=(y~*rstd)*z, final fused
     (cv2 @ diag(scale) @ out_proj @ diag(ln_g)) matmul -> delta bf16.
Host: shards/reassembles, transposes, premultiplies u*B, merges
  h_f*C_f + h_r*C_r + D*xc pairs, adds residual x + cv2 bias.
"""
import os
import sys

sys.path.insert(0, '/opt/trn_rl_repo')

import numpy as np
import ml_dtypes

import concourse.bass as bass
import concourse.tile as tile
import concourse.mybir as mybir
from concourse.bass_utils import run_bass_kernel_spmd

bf16 = mybir.dt.bfloat16
fp8 = mybir.dt.float8e4
f32 = mybir.dt.float32
NF8 = ml_dtypes.float8_e4m3
DR = mybir.MatmulPerfMode.DoubleRow
FS = 32.0  # fp8 fold-weight prescale (values ~0.01 would be subnormal in e4m3)
MULT, ADD = mybir.AluOpType.mult, mybir.AluOpType.add
SUB = mybir.AluOpType.subtract
AF = mybir.ActivationFunctionType
NBF = ml_dtypes.bfloat16

B, C1, C2, H, W = 4, 256, 256, 128, 128
Cm, K, R = 128, 4, 8
L = H * W          # 16384
HH = H // 2        # 64 rows per half
LH = HH * W        # 8192
CH = 2048          # L2 chunk
NCH = L // CH      # 8
WU = 64            # scan warmup columns
LP = L + 2 * WU    # padded length 16512

EXEC_TIMES = {}    # launch -> exec ns (MAMBA_TRACE=1)
_CACHE = {}


def _split_multiwaits(nc):
    """walrus here accepts ONE sync-wait per instruction; hoist extras into
    single-wait same-engine NOPs inserted before the instruction."""
    for f in nc.m.functions:
        for bb in f.blocks:
            il = bb.instructions
            i = 0
            while i < len(il):
                ins = il[i]
                si = getattr(ins, "sync_info", None)
                if si is not None and len(si.on_wait) > 1:
                    waits = list(si.on_wait)
                    ins.sync_info = mybir.SyncInfo(
                        on_wait=[waits[-1]], on_update=list(si.on_update))
                    for w in waits[:-1]:
                        nop = mybir.InstNoOp(
                            name=nc.get_next_instruction_name(), ins=[], outs=[])
                        nop.engine = ins.engine
                        nop.sync_info = mybir.SyncInfo(on_wait=[w], on_update=[])
                        nc.register_instruction(nop, overwrite=True)
                        il.insert(i, nop)
                        i += 1
                i += 1


def _new_nc():
    return bass.Bass("TRN2", target_bir_lowering=False, debug=False,
                     enable_asserts=True, num_devices=8)


def _run(nc, in_maps, name):
    trace = os.environ.get("MAMBA_TRACE", "0") == "1"
    res = run_bass_kernel_spmd(nc, in_maps, core_ids=list(range(8)), trace=trace)
    if trace:
        EXEC_TIMES[name] = res.exec_time_ns
    return res.results


# ------------------------------------------------------------------- L1
def build_l1():
    nc = _new_nc()
    x_in = nc.dram_tensor("x_in", [C1, HH + 2, W], bf16, kind="ExternalInput")
    wcv1 = nc.dram_tensor("wcv1", [C1, Cm], bf16, kind="ExternalInput")       # lhsT
    bcv1 = nc.dram_tensor("bcv1", [Cm, 1], f32, kind="ExternalInput")
    wfold = nc.dram_tensor("wfold", [Cm, 9, Cm], bf16, kind="ExternalInput")  # (k, tap, m)
    bconv = nc.dram_tensor("bconv", [Cm, 1], f32, kind="ExternalInput")
    wz = nc.dram_tensor("wz", [Cm, Cm], bf16, kind="ExternalInput")           # lhsT
    wbc = nc.dram_tensor("wbc", [Cm, 8], bf16, kind="ExternalInput")          # lhsT
    hmask = nc.dram_tensor("hmask", [Cm, 2], f32, kind="ExternalInput")
    xc_out = nc.dram_tensor("xc_out", [Cm, LH], bf16, kind="ExternalOutput")
    z_out = nc.dram_tensor("z_out", [Cm, LH], bf16, kind="ExternalOutput")
    bcr_out = nc.dram_tensor("bcr_out", [8, LH], bf16, kind="ExternalOutput")

    HP = HH + 2   # 66
    WP = W + 2    # 130

    with tile.TileContext(nc) as tc, \
         tc.tile_pool(name="w", bufs=1) as wp, \
         tc.tile_pool(name="d", bufs=1) as dp, \
         tc.tile_pool(name="ps", bufs=2, space="PSUM") as pp, \
         tc.tile_pool(name="ps8", bufs=2, space="PSUM") as pp8:
        tw1a = wp.tile([128, Cm], bf16)
        tw1b = wp.tile([128, Cm], bf16)
        nc.sync.dma_start(out=tw1a, in_=wcv1[0:128, :])
        nc.sync.dma_start(out=tw1b, in_=wcv1[128:256, :])
        twf = wp.tile([Cm, 9, Cm], bf16)
        nc.sync.dma_start(out=twf, in_=wfold[:, :, :])
        twz = wp.tile([Cm, Cm], bf16)
        nc.sync.dma_start(out=twz, in_=wz[:, :])
        twbc = wp.tile([Cm, 8], bf16)
        nc.sync.dma_start(out=twbc, in_=wbc[:, :])
        tb1 = wp.tile([Cm, 1], f32)
        nc.sync.dma_start(out=tb1, in_=bcv1[:, :])
        tbc = wp.tile([Cm, 1], f32)
        nc.sync.dma_start(out=tbc, in_=bconv[:, :])
        tmask = wp.tile([Cm, 2], f32)
        nc.sync.dma_start(out=tmask, in_=hmask[:, :])

        txa = dp.tile([128, HP, W], bf16)
        txb = dp.tile([128, HP, W], bf16)
        for rb in range(0, HP, 11):
            nc.sync.dma_start(out=txa[:, rb:rb + 11, :], in_=x_in[0:128, rb:rb + 11, :])
            nc.sync.dma_start(out=txb[:, rb:rb + 11, :], in_=x_in[128:256, rb:rb + 11, :])

        th = dp.tile([Cm, HP, WP], bf16)
        nc.vector.memset(th[:, :, 0:1], 0.0)
        nc.vector.memset(th[:, :, WP - 1:WP], 0.0)

        # cv1 over 66 rows: 16 chunks of 4 rows + 1 chunk of 2 rows
        row_chunks = [(r0, 4) for r0 in range(0, 64, 4)] + [(64, 2)]
        for r0, nr in row_chunks:
            pt = pp.tile([Cm, 512], f32, tag="cv1")
            nn = nr * W
            nc.tensor.matmul(out=pt[:, :nn], lhsT=tw1a[:, :],
                             rhs=txa[:, r0:r0 + nr, :], start=True, stop=False)
            nc.tensor.matmul(out=pt[:, :nn], lhsT=tw1b[:, :],
                             rhs=txb[:, r0:r0 + nr, :], start=False, stop=True)
            nc.scalar.activation(out=th[:, r0:r0 + nr, 1:W + 1], in_=pt[:, :nn],
                                 func=AF.Identity, bias=tb1[:, :], scale=1.0)
        nc.vector.tensor_scalar_mul(out=th[:, 0, :], in0=th[:, 0, :],
                                    scalar1=tmask[:, 0:1])
        nc.vector.tensor_scalar_mul(out=th[:, HP - 1, :], in0=th[:, HP - 1, :],
                                    scalar1=tmask[:, 1:2])
        txc = dp.tile([Cm, HH, W], bf16)
        tz = dp.tile([Cm, HH, W], bf16)
        tbcr = dp.tile([8, LH], bf16)
        for r0 in range(0, HH, 4):
            pt = pp.tile([Cm, 512], f32, tag="fold")
            for t in range(9):
                dy, dx = t // 3 - 1, t % 3 - 1
                nc.tensor.matmul(
                    out=pt[:, :], lhsT=twf[:, t, :],
                    rhs=th[:, r0 + 1 + dy:r0 + 5 + dy, 1 + dx:W + 1 + dx],
                    start=(t == 0), stop=(t == 8))
            nc.scalar.activation(out=txc[:, r0:r0 + 4, :], in_=pt[:, :],
                                 func=AF.Silu, bias=tbc[:, :], scale=1.0)
            ptz = pp.tile([Cm, 512], f32, tag="z")
            nc.tensor.matmul(out=ptz[:, :], lhsT=twz[:, :],
                             rhs=th[:, r0 + 1:r0 + 5, 1:W + 1],
                             start=True, stop=True)
            nc.scalar.activation(out=tz[:, r0:r0 + 4, :], in_=ptz[:, :],
                                 func=AF.Silu, bias=0.0, scale=1.0)
            ptb = pp8.tile([8, 512], f32, tag="bc")
            nc.tensor.matmul(out=ptb[:, :], lhsT=twbc[:, :],
                             rhs=txc[:, r0:r0 + 4, :], start=True, stop=True)
            nc.vector.tensor_copy(out=tbcr[:, r0 * W:(r0 + 4) * W], in_=ptb[:, :])

            if r0 % 16 == 12:  # flush every 16 rows
                rs = r0 - 12
                nc.sync.dma_start(out=xc_out[:, rs * W:(r0 + 4) * W],
                                  in_=txc[:, rs:r0 + 4, :])
                nc.sync.dma_start(out=z_out[:, rs * W:(r0 + 4) * W],
                                  in_=tz[:, rs:r0 + 4, :])
        nc.sync.dma_start(out=bcr_out[:, :], in_=tbcr[:, :])
    return nc


# ------------------------------------------------------------------- L2
# tensor_tensor_scan only runs on DVE (compiler rejects it on Pool), and
# Pool tensor_tensor computes garbage on HW, so all elementwise stays on
# DVE. Reversed-AP scans cost ~2x, so the host pre-reverses the reverse
# direction's arrays and both directions scan forward here.


def build_l2():
    nc = _new_nc()
    u_f = nc.dram_tensor("u_f", [Cm, LP], bf16, kind="ExternalInput")
    u_r = nc.dram_tensor("u_r", [Cm, LP], bf16, kind="ExternalInput")
    ub_f = nc.dram_tensor("ub_f", [Cm, LP], bf16, kind="ExternalInput")
    ub_r = nc.dram_tensor("ub_r", [Cm, LP], bf16, kind="ExternalInput")
    wdt_f = nc.dram_tensor("wdt_f", [Cm, Cm], bf16, kind="ExternalInput")
    wdt_r = nc.dram_tensor("wdt_r", [Cm, Cm], bf16, kind="ExternalInput")
    dtb_f = nc.dram_tensor("dtb_f", [Cm, 1], f32, kind="ExternalInput")
    dtb_r = nc.dram_tensor("dtb_r", [Cm, 1], f32, kind="ExternalInput")
    a_f = nc.dram_tensor("a_f", [Cm, 1], f32, kind="ExternalInput")
    a_r = nc.dram_tensor("a_r", [Cm, 1], f32, kind="ExternalInput")
    hf_out = nc.dram_tensor("hf_out", [Cm, L], bf16, kind="ExternalOutput")
    hr_out = nc.dram_tensor("hr_out", [Cm, L], bf16, kind="ExternalOutput")

    EXT = CH + WU  # 2112
    with tile.TileContext(nc) as tc, \
         tc.tile_pool(name="w", bufs=1) as wp, \
         tc.tile_pool(name="u", bufs=1) as up, \
         tc.tile_pool(name="e1", bufs=2) as e1p, \
         tc.tile_pool(name="av", bufs=2) as avp, \
         tc.tile_pool(name="bt", bufs=2) as btp, \
         tc.tile_pool(name="h", bufs=3) as hp, \
         tc.tile_pool(name="psa", bufs=2, space="PSUM") as psa, \
         tc.tile_pool(name="psb", bufs=1, space="PSUM") as psb:
        twf_ = wp.tile([Cm, Cm], bf16)
        twr_ = wp.tile([Cm, Cm], bf16)
        nc.sync.dma_start(out=twf_, in_=wdt_f[:, :])
        nc.sync.dma_start(out=twr_, in_=wdt_r[:, :])
        tbf = wp.tile([Cm, 1], f32)
        tbr = wp.tile([Cm, 1], f32)
        taf = wp.tile([Cm, 1], f32)
        tar = wp.tile([Cm, 1], f32)
        nc.sync.dma_start(out=tbf, in_=dtb_f[:, :])
        nc.sync.dma_start(out=tbr, in_=dtb_r[:, :])
        nc.sync.dma_start(out=taf, in_=a_f[:, :])
        nc.sync.dma_start(out=tar, in_=a_r[:, :])

        tuf = up.tile([Cm, LP], bf16)
        tur = up.tile([Cm, LP], bf16)
        tubf = up.tile([Cm, LP], bf16)
        tubr = up.tile([Cm, LP], bf16)
        # first slice = exactly chunk 0's span so compute starts ASAP;
        # fwd arrays first (chunk f0 is issued first)
        SLR = (LP - EXT) // 6  # 2400
        bounds = [0, EXT] + [EXT + SLR * (i + 1) for i in range(6)]
        arrs = [(tuf, u_f), (tubf, ub_f), (tur, u_r), (tubr, ub_r)]
        for t, d in arrs:
            s = slice(0, EXT)
            nc.sync.dma_start(out=t[:, s], in_=d[:, s])
        for i in range(1, 7):
            s = slice(bounds[i], bounds[i + 1])
            for t, d in arrs:
                nc.sync.dma_start(out=t[:, s], in_=d[:, s])

        dirs = {
            'f': (twf_, tbf, taf, tuf, tubf, hf_out),
            'r': (twr_, tbr, tar, tur, tubr, hr_out),
        }
        PIECES = [(0, 1024), (1024, 1088)]

        def chunk(d, ci):
            tw, tb, ta, tu, tub, hout = dirs[d]
            sp0 = ci * CH  # padded-coords span start, len EXT
            # softplus via exp->ln (both in the natural_log_exp ACT table)
            te1 = e1p.tile([Cm, EXT], bf16, tag="e1" + d)
            for po, pl in PIECES:
                pool = psa if po == 0 else psb
                pt = pool.tile([Cm, pl], f32, tag="dtd%d" % po)
                for so in range(0, pl, 512):
                    sl_len = min(512, pl - so)
                    nc.tensor.matmul(
                        out=pt[:, so:so + sl_len], lhsT=tw[:, :],
                        rhs=tu[:, sp0 + po + so: sp0 + po + so + sl_len],
                        start=True, stop=True)
                nc.scalar.activation(out=te1[:, po:po + pl], in_=pt[:, 0:pl],
                                     func=AF.Exp, bias=tb[:, :], scale=1.0)
            # dt = ln(1 + e1) in place: te1 becomes dt
            nc.scalar.activation(out=te1, in_=te1, func=AF.Ln,
                                 bias=1.0, scale=1.0)
            tdt = te1
            tav = avp.tile([Cm, EXT], bf16, tag="av" + d)
            nc.scalar.activation(out=tav, in_=tdt, func=AF.Exp,
                                 bias=0.0, scale=ta[:, :])
            tbt = btp.tile([Cm, EXT], bf16, tag="bt" + d)
            nc.vector.tensor_tensor(out=tbt, in0=tdt,
                                    in1=tub[:, sp0:sp0 + EXT], op=MULT)
            th = hp.tile([Cm, EXT], bf16, tag="h" + d)
            nc.vector.tensor_tensor_scan(out=th, data0=tav, data1=tbt,
                                         initial=0.0, op0=MULT, op1=ADD)
            # h-out triggers wait on the scan; keep them OFF the ACT queue
            # (an ACT-issued DMA wait would head-block later ln/exp ops).
            nc.sync.dma_start(out=hout[:, ci * CH:(ci + 1) * CH], in_=th[:, WU:EXT])

        for ci in range(NCH):
            chunk('f', ci)
            chunk('r', ci)
    return nc


# ------------------------------------------------------------------- L3
def build_l3(with_lnb):
    nc = _new_nc()
    y_in = nc.dram_tensor("y_in", [Cm, LH], bf16, kind="ExternalInput")
    z_in = nc.dram_tensor("z_in", [Cm, LH], bf16, kind="ExternalInput")
    wcen = nc.dram_tensor("wcen", [Cm, Cm], bf16, kind="ExternalInput")   # I - J/128
    wvar = nc.dram_tensor("wvar", [Cm, Cm], bf16, kind="ExternalInput")   # J/128
    wfin = nc.dram_tensor("wfin", [Cm, C2], bf16, kind="ExternalInput")   # lhsT
    wfinb = nc.dram_tensor("wfinb", [Cm, C2], bf16, kind="ExternalInput") # lhsT (ln_b path)
    d_out = nc.dram_tensor("d_out", [C2, LH], bf16, kind="ExternalOutput")

    NC3 = LH // 512  # 16
    with tile.TileContext(nc) as tc, \
         tc.tile_pool(name="w", bufs=1) as wp, \
         tc.tile_pool(name="d", bufs=1) as dp, \
         tc.tile_pool(name="c", bufs=4) as cp, \
         tc.tile_pool(name="pc", bufs=3, space="PSUM") as ppc, \
         tc.tile_pool(name="pv", bufs=2, space="PSUM") as ppv, \
         tc.tile_pool(name="pd", bufs=3, space="PSUM") as ppd:
        twc = wp.tile([Cm, Cm], bf16)
        nc.sync.dma_start(out=twc, in_=wcen[:, :])
        twv = wp.tile([Cm, Cm], bf16)
        nc.sync.dma_start(out=twv, in_=wvar[:, :])
        twa = wp.tile([Cm, 128], bf16)
        twb = wp.tile([Cm, 128], bf16)
        nc.scalar.dma_start(out=twa, in_=wfin[:, 0:128])
        nc.scalar.dma_start(out=twb, in_=wfin[:, 128:256])
        if with_lnb:
            twba = wp.tile([Cm, 128], bf16)
            twbb = wp.tile([Cm, 128], bf16)
            nc.scalar.dma_start(out=twba, in_=wfinb[:, 0:128])
            nc.scalar.dma_start(out=twbb, in_=wfinb[:, 128:256])

        teps = wp.tile([Cm, 1], f32)
        nc.vector.memset(teps, 1e-5)

        ty = dp.tile([Cm, LH], bf16)
        tz = dp.tile([Cm, LH], bf16)
        for i in range(4):
            s = slice(i * LH // 4, (i + 1) * LH // 4)
            nc.sync.dma_start(out=ty[:, s], in_=y_in[:, s])
            nc.sync.dma_start(out=tz[:, s], in_=z_in[:, s])

        # software-pipelined issue: stagger stages across chunks so each
        # in-order engine queue (esp. ACT) never head-blocks on a fresh dep.
        pcs, pvs, tts = {}, {}, {}

        def stage0(k):
            s = slice(k * 512, (k + 1) * 512)
            pc = ppc.tile([Cm, 512], f32, tag="c")
            pcs[k] = pc
            nc.tensor.matmul(out=pc, lhsT=twc[:, :], rhs=ty[:, s],
                             start=True, stop=True)
            tsq = cp.tile([Cm, 512], bf16, tag="sq")
            nc.scalar.activation(out=tsq, in_=pc, func=AF.Square,
                                 bias=0.0, scale=1.0)
            pv = ppv.tile([Cm, 512], f32, tag="v")
            pvs[k] = pv
            nc.tensor.matmul(out=pv, lhsT=twv[:, :], rhs=tsq,
                             start=True, stop=True)

        def stage1(k):
            s = slice(k * 512, (k + 1) * 512)
            tln = cp.tile([Cm, 512], f32, tag="ln")
            nc.scalar.activation(out=tln, in_=pvs.pop(k), func=AF.Ln,
                                 bias=teps[:, :], scale=1.0)
            trs = cp.tile([Cm, 512], bf16, tag="rs")
            nc.scalar.activation(out=trs, in_=tln, func=AF.Exp,
                                 bias=0.0, scale=-0.5)
            trz = cp.tile([Cm, 512], bf16, tag="rz")
            nc.vector.tensor_tensor(out=trz, in0=trs, in1=tz[:, s], op=MULT)
            tt = cp.tile([Cm, 512], bf16, tag="t")
            tts[k] = tt
            nc.vector.tensor_tensor(out=tt, in0=pcs.pop(k), in1=trz, op=MULT)

        obuf = {}

        def stage2(k):
            s = slice(k * 512, (k + 1) * 512)
            tt = tts.pop(k)
            pda = ppd.tile([Cm, 512], f32, tag="d")
            nc.tensor.matmul(out=pda, lhsT=twa[:, :], rhs=tt,
                             start=True, stop=not with_lnb)
            if with_lnb:
                nc.tensor.matmul(out=pda, lhsT=twba[:, :], rhs=tz[:, s],
                                 start=False, stop=True)
            pdb = ppd.tile([Cm, 512], f32, tag="d")
            nc.tensor.matmul(out=pdb, lhsT=twb[:, :], rhs=tt,
                             start=True, stop=not with_lnb)
            if with_lnb:
                nc.tensor.matmul(out=pdb, lhsT=twbb[:, :], rhs=tz[:, s],
                                 start=False, stop=True)
            # batch output DMAs in chunk pairs to halve sync-queue triggers
            if k % 2 == 0:
                toa = cp.tile([128, 1024], bf16, tag="oa")
                tob = cp.tile([128, 1024], bf16, tag="ob")
                obuf['a'], obuf['b'] = toa, tob
            off = (k % 2) * 512
            nc.vector.tensor_copy(out=obuf['a'][:, off:off + 512], in_=pda)
            nc.vector.tensor_copy(out=obuf['b'][:, off:off + 512], in_=pdb)
            if k % 2 == 1:
                sp = slice((k - 1) * 512, (k + 1) * 512)
                nc.sync.dma_start(out=d_out[0:128, sp], in_=obuf['a'])
                nc.sync.dma_start(out=d_out[128:256, sp], in_=obuf['b'])

        for k in range(NC3 + 2):
            if k < NC3:
                stage0(k)
            if 1 <= k:
                if k - 1 < NC3:
                    stage1(k - 1)
            if k >= 2:
                stage2(k - 2)
    return nc


# ------------------------------------------------------------------- host
def _get_ncs(with_lnb):
    key = ("ncs", with_lnb)
    if key not in _CACHE:
        nc1, nc2, nc3 = build_l1(), build_l2(), build_l3(with_lnb)
        for n in (nc1, nc2, nc3):
            _split_multiwaits(n)
        _CACHE[key] = (nc1, nc2, nc3)
    return _CACHE[key]


def kernel(x, cv1_w, cv1_b, scale_w, in_proj_w, conv_w, conv_b, x_proj_w,
           dt_w, dt_b, A_logs, Ds, ln_g, ln_b, out_proj_w, cv2_w, cv2_b):
    f = np.float32
    x = np.asarray(x, f)
    cv1_w = np.asarray(cv1_w, f); cv1_b = np.asarray(cv1_b, f)
    in_proj_w = np.asarray(in_proj_w, f)
    conv_w = np.asarray(conv_w, f); conv_b = np.asarray(conv_b, f)
    x_proj_w = np.asarray(x_proj_w, f)
    dt_w = np.asarray(dt_w, f); dt_b = np.asarray(dt_b, f)
    A_logs = np.asarray(A_logs, f); Ds = np.asarray(Ds, f)
    ln_g = np.asarray(ln_g, f); ln_b = np.asarray(ln_b, f)
    out_proj_w = np.asarray(out_proj_w, f)
    cv2_w = np.asarray(cv2_w, f); cv2_b = np.asarray(cv2_b, f)
    scale_v = np.asarray(scale_w, f).reshape(Cm)

    Wip_x, Wip_z = in_proj_w[:Cm], in_proj_w[Cm:]
    dwk = conv_w[:, 0]
    A = -np.exp(A_logs).reshape(K, Cm)
    Dk = Ds.reshape(K, Cm)
    Dall = Dk.sum(axis=0)
    W_dtk = np.einsum('kdr,krc->kdc', dt_w, x_proj_w[:, :R])
    WB, WC = x_proj_w[:, R], x_proj_w[:, R + 1]
    W_final = cv2_w @ (scale_v[:, None] * out_proj_w)
    W1 = W_final * ln_g[None, :]
    with_lnb = bool(np.any(ln_b != 0.0))
    Wb1 = W_final * ln_b[None, :]

    # fold lhsT: (tap, k=h-chan, m=out-chan) -> host layout (k, tap, m)
    Wfold = np.einsum('cyx,cd->yxdc', dwk, Wip_x)      # (3,3, in, out)
    wfold_rm = np.ascontiguousarray(
        Wfold.reshape(9, Cm, Cm).transpose(1, 0, 2))   # row-major cores
    wbc_l = np.stack([WB[0], WC[0], WB[2], WC[2],
                      WB[1], WC[1], WB[3], WC[3]], axis=1)

    nc1, nc2, nc3 = _get_ncs(with_lnb)

    # ---------------- L1 ----------------
    l1_maps = []
    wcv1_h = np.ascontiguousarray(cv1_w.T).astype(NBF)
    wfold_h = wfold_rm.astype(NBF)
    wz_h = np.ascontiguousarray(Wip_z.T).astype(NBF)
    wbc_h = np.ascontiguousarray(wbc_l).astype(NBF)
    for core in range(8):
        b, half = core // 2, core % 2
        r0 = half * HH
        xs = np.zeros((C1, HH + 2, W), np.float32)
        lo, hi = r0 - 1, r0 + HH + 1
        slo, shi = max(lo, 0), min(hi, H)
        xs[:, slo - lo: shi - lo, :] = x[b, :, slo:shi, :]
        mask = np.ones((Cm, 2), np.float32)
        mask[:, 0] = 0.0 if half == 0 else 1.0
        mask[:, 1] = 1.0 if half == 0 else 0.0
        l1_maps.append({
            "x_in": xs.astype(NBF),
            "wcv1": wcv1_h,
            "bcv1": cv1_b.reshape(Cm, 1),
            "wfold": wfold_h,
            "bconv": conv_b.reshape(Cm, 1),
            "wz": wz_h,
            "wbc": wbc_h,
            "hmask": mask,
        })
    r1 = _run(nc1, l1_maps, "L1")

    xc = np.zeros((B, Cm, L), NBF)
    zf = np.zeros((B, Cm, L), NBF)
    rows = np.zeros((B, 8, L), f)
    for core in range(8):
        b, half = core // 2, core % 2
        sl = slice(half * LH, (half + 1) * LH)
        xc[b][:, sl] = r1[core]["xc_out"]
        zf[b][:, sl] = r1[core]["z_out"]
        rows[b][:, sl] = r1[core]["bcr_out"].astype(f)

    # ---------------- L2 ----------------
    def t_spatial(a):
        return np.ascontiguousarray(
            a.reshape(*a.shape[:-1], H, W).swapaxes(-1, -2).reshape(*a.shape[:-1], L))

    l2_maps = []
    crows = []  # per core (Cf, Cr) for host merge
    for core in range(8):
        b, g = core // 2, core % 2
        if g == 0:
            u = xc[b].astype(f)
            kf, kr = 0, 2
            br_f, cr_f = rows[b][0], rows[b][1]
            br_r, cr_r = rows[b][2], rows[b][3]
        else:
            u = t_spatial(xc[b]).astype(f)
            kf, kr = 1, 3
            br_f, cr_f = t_spatial(rows[b][4]), t_spatial(rows[b][5])
            br_r, cr_r = t_spatial(rows[b][6]), t_spatial(rows[b][7])
        upf = np.zeros((Cm, LP), f)
        upf[:, WU:WU + L] = u
        upr = np.zeros((Cm, LP), f)
        upr[:, WU:WU + L] = u[:, ::-1]
        ubf = np.zeros((Cm, LP), f)
        ubf[:, WU:WU + L] = u * br_f[None, :]
        ubr = np.zeros((Cm, LP), f)
        ubr[:, WU:WU + L] = (u * br_r[None, :])[:, ::-1]
        crows.append((cr_f, cr_r))
        l2_maps.append({
            "u_f": upf.astype(NBF),
            "u_r": upr.astype(NBF),
            "ub_f": ubf.astype(NBF),
            "ub_r": ubr.astype(NBF),
            "wdt_f": np.ascontiguousarray(W_dtk[kf].T).astype(NBF),
            "wdt_r": np.ascontiguousarray(W_dtk[kr].T).astype(NBF),
            "dtb_f": dt_b[kf].reshape(Cm, 1), "dtb_r": dt_b[kr].reshape(Cm, 1),
            "a_f": A[kf].reshape(Cm, 1).astype(f), "a_r": A[kr].reshape(Cm, 1).astype(f),
        })
    r2 = _run(nc2, l2_maps, "L2")

    # host merge: m_g = hf*Cf + hr*Cr ; y = m_0 + t(m_1) + Dall*xc
    l3_maps = []
    wcen_h = (np.eye(Cm, dtype=f) - np.full((Cm, Cm), 1.0 / Cm, f)).astype(NBF)
    wvar_h = np.full((Cm, Cm), 1.0 / Cm, f).astype(NBF)
    wfin_h = np.ascontiguousarray(W1.T).astype(NBF)
    wfinb_h = np.ascontiguousarray(Wb1.T).astype(NBF)
    for b in range(B):
        cf0, cr0 = crows[2 * b]
        cf1, cr1 = crows[2 * b + 1]
        m0 = (r2[2 * b]["hf_out"].astype(f) * cf0[None, :]
              + r2[2 * b]["hr_out"][:, ::-1].astype(f) * cr0[None, :])
        m1 = (r2[2 * b + 1]["hf_out"].astype(f) * cf1[None, :]
              + r2[2 * b + 1]["hr_out"][:, ::-1].astype(f) * cr1[None, :])
        ypre = (m0 + t_spatial(m1) + Dall[:, None] * xc[b].astype(f)).astype(NBF)
        for half in range(2):
            sl = slice(half * LH, (half + 1) * LH)
            l3_maps.append({
                "y_in": np.ascontiguousarray(ypre[:, sl]),
                "z_in": np.ascontiguousarray(zf[b][:, sl]),
                "wcen": wcen_h,
                "wvar": wvar_h,
                "wfin": wfin_h,
                "wfinb": wfinb_h,
            })
    r3 = _run(nc3, l3_maps, "L3")

    out = np.empty((B, C2, H, W), np.float32)
    for core in range(8):
        b, half = core // 2, core % 2
        sl = slice(half * LH, (half + 1) * LH)
        out[b].reshape(C2, L)[:, sl] = r3[core]["d_out"].astype(f)
    out += x
    out += cv2_b[None, :, None, None]
    return out


# revision 61
# speedup vs baseline: 1.1957x; 1.0108x over previous
"""BottleneckMamba Trainium2 kernel (self-contained).

out = x + cv2( scale * out_proj( LN(cross-merge(4-dir selective scan(N=1))) * z ) )

3 SPMD launches on 8 NeuronCores:
  L1 (core=(b, image-half)): cv1 -> h; depthwise3x3*in_proj folded into 9
     matmuls -> silu -> xc ; z = silu(Wz@h) ; B/C projection rows.
  L2 (core=(b, dir-group)): per direction (fwd/rev over its u layout):
     dtd matmul -> softplus (ACT) -> av=exp(A*dt) (ACT) -> bt=dt*ub (DVE,
     ub=u*B premultiplied on host) -> warmup-window scans (chunks are
     independent: state influence decays below 1e-14 within 64 cols, so
     each 2048-chunk scans [chunk-64, chunk_end) from state 0) split
     across DVE and Pool engines. Raw h written out per direction.
  L3 (core=(b, half)): y centered via (I-J/128) matmul, var via (J/128)
     matmul of y~^2, rstd on ACT, t=(y~*rstd)*z, final fused
     (cv2 @ diag(scale) @ out_proj @ diag(ln_g)) matmul -> delta bf16.
Host: shards/reassembles, transposes, premultiplies u*B, merges
  h_f*C_f + h_r*C_r + D*xc pairs, adds residual x + cv2 bias.
"""
import os
import sys

sys.path.insert(0, '/opt/trn_rl_repo')

import numpy as np
import ml_dtypes

import concourse.bass as bass
import concourse.tile as tile
import concourse.mybir as mybir
from concourse.bass_utils import run_bass_kernel_spmd

bf16 = mybir.dt.bfloat16
fp8 = mybir.dt.float8e4
f32 = mybir.dt.float32
NF8 = ml_dtypes.float8_e4m3
DR = mybir.MatmulPerfMode.DoubleRow
FS = 32.0  # fp8 fold-weight prescale (values ~0.01 would be subnormal in e4m3)
MULT, ADD = mybir.AluOpType.mult, mybir.AluOpType.add
SUB = mybir.AluOpType.subtract
AF = mybir.ActivationFunctionType
NBF = ml_dtypes.bfloat16

B, C1, C2, H, W = 4, 256, 256, 128, 128
Cm, K, R = 128, 4, 8
L = H * W          # 16384
HH = H // 2        # 64 rows per half
LH = HH * W        # 8192
CH = 2048          # L2 chunk
NCH = L // CH      # 8
WU = 64            # scan warmup columns
LP = L + 2 * WU    # padded length 16512

EXEC_TIMES = {}    # launch -> exec ns (MAMBA_TRACE=1)
_CACHE = {}


def _split_multiwaits(nc):
    """walrus here accepts ONE sync-wait per instruction; hoist extras into
    single-wait same-engine NOPs inserted before the instruction."""
    for f in nc.m.functions:
        for bb in f.blocks:
            il = bb.instructions
            i = 0
            while i < len(il):
                ins = il[i]
                si = getattr(ins, "sync_info", None)
                if si is not None and len(si.on_wait) > 1:
                    waits = list(si.on_wait)
                    ins.sync_info = mybir.SyncInfo(
                        on_wait=[waits[-1]], on_update=list(si.on_update))
                    for w in waits[:-1]:
                        nop = mybir.InstNoOp(
                            name=nc.get_next_instruction_name(), ins=[], outs=[])
                        nop.engine = ins.engine
                        nop.sync_info = mybir.SyncInfo(on_wait=[w], on_update=[])
                        nc.register_instruction(nop, overwrite=True)
                        il.insert(i, nop)
                        i += 1
                i += 1


def _new_nc():
    return bass.Bass("TRN2", target_bir_lowering=False, debug=False,
                     enable_asserts=True, num_devices=8)


def _run(nc, in_maps, name):
    trace = os.environ.get("MAMBA_TRACE", "0") == "1"
    res = run_bass_kernel_spmd(nc, in_maps, core_ids=list(range(8)), trace=trace)
    if trace:
        EXEC_TIMES[name] = res.exec_time_ns
    return res.results


# ------------------------------------------------------------------- L1
def build_l1():
    nc = _new_nc()
    x_in = nc.dram_tensor("x_in", [C1, HH + 2, W], bf16, kind="ExternalInput")
    wcv1 = nc.dram_tensor("wcv1", [C1, Cm], bf16, kind="ExternalInput")       # lhsT
    bcv1 = nc.dram_tensor("bcv1", [Cm, 1], f32, kind="ExternalInput")
    wfold = nc.dram_tensor("wfold", [Cm, 9, Cm], bf16, kind="ExternalInput")  # (k, tap, m)
    bconv = nc.dram_tensor("bconv", [Cm, 1], f32, kind="ExternalInput")
    wz = nc.dram_tensor("wz", [Cm, Cm], bf16, kind="ExternalInput")           # lhsT
    wbc = nc.dram_tensor("wbc", [Cm, 8], bf16, kind="ExternalInput")          # lhsT
    hmask = nc.dram_tensor("hmask", [Cm, 2], f32, kind="ExternalInput")
    xc_out = nc.dram_tensor("xc_out", [Cm, LH], bf16, kind="ExternalOutput")
    z_out = nc.dram_tensor("z_out", [Cm, LH], bf16, kind="ExternalOutput")
    bcr_out = nc.dram_tensor("bcr_out", [8, LH], bf16, kind="ExternalOutput")

    HP = HH + 2   # 66
    WP = W + 2    # 130

    with tile.TileContext(nc) as tc, \
         tc.tile_pool(name="w", bufs=1) as wp, \
         tc.tile_pool(name="d", bufs=1) as dp, \
         tc.tile_pool(name="ps", bufs=2, space="PSUM") as pp, \
         tc.tile_pool(name="ps8", bufs=2, space="PSUM") as pp8:
        tw1a = wp.tile([128, Cm], bf16)
        tw1b = wp.tile([128, Cm], bf16)
        nc.sync.dma_start(out=tw1a, in_=wcv1[0:128, :])
        nc.sync.dma_start(out=tw1b, in_=wcv1[128:256, :])
        twf = wp.tile([Cm, 9, Cm], bf16)
        nc.sync.dma_start(out=twf, in_=wfold[:, :, :])
        twz = wp.tile([Cm, Cm], bf16)
        nc.sync.dma_start(out=twz, in_=wz[:, :])
        twbc = wp.tile([Cm, 8], bf16)
        nc.sync.dma_start(out=twbc, in_=wbc[:, :])
        tb1 = wp.tile([Cm, 1], f32)
        nc.sync.dma_start(out=tb1, in_=bcv1[:, :])
        tbc = wp.tile([Cm, 1], f32)
        nc.sync.dma_start(out=tbc, in_=bconv[:, :])
        tmask = wp.tile([Cm, 2], f32)
        nc.sync.dma_start(out=tmask, in_=hmask[:, :])

        twarm = wp.tile([128, 1], bf16)
        nc.vector.memset(twarm, 0.0)
        nc.scalar.activation(out=twarm, in_=twarm, func=AF.Silu,
                             bias=0.0, scale=1.0)

        txa = dp.tile([128, HP, W], bf16)
        txb = dp.tile([128, HP, W], bf16)
        xrow_chunks = [(0, 4), (4, 7)] + [(rb, 11) for rb in range(11, HP, 11)]
        for rb, nrb in xrow_chunks:
            nc.sync.dma_start(out=txa[:, rb:rb + nrb, :], in_=x_in[0:128, rb:rb + nrb, :])
            nc.sync.dma_start(out=txb[:, rb:rb + nrb, :], in_=x_in[128:256, rb:rb + nrb, :])

        th = dp.tile([Cm, HP, WP], bf16)
        nc.vector.memset(th[:, :, 0:1], 0.0)
        nc.vector.memset(th[:, :, WP - 1:WP], 0.0)

        # cv1 over 66 rows: 16 chunks of 4 rows + 1 chunk of 2 rows
        row_chunks = [(r0, 4) for r0 in range(0, 64, 4)] + [(64, 2)]
        for r0, nr in row_chunks:
            pt = pp.tile([Cm, 512], f32, tag="cv1")
            nn = nr * W
            nc.tensor.matmul(out=pt[:, :nn], lhsT=tw1a[:, :],
                             rhs=txa[:, r0:r0 + nr, :], start=True, stop=False)
            nc.tensor.matmul(out=pt[:, :nn], lhsT=tw1b[:, :],
                             rhs=txb[:, r0:r0 + nr, :], start=False, stop=True)
            nc.scalar.activation(out=th[:, r0:r0 + nr, 1:W + 1], in_=pt[:, :nn],
                                 func=AF.Identity, bias=tb1[:, :], scale=1.0)
        nc.vector.tensor_scalar_mul(out=th[:, 0, :], in0=th[:, 0, :],
                                    scalar1=tmask[:, 0:1])
        nc.vector.tensor_scalar_mul(out=th[:, HP - 1, :], in0=th[:, HP - 1, :],
                                    scalar1=tmask[:, 1:2])
        txc = dp.tile([Cm, HH, W], bf16)
        tz = dp.tile([Cm, HH, W], bf16)
        tbcr = dp.tile([8, LH], bf16)
        for r0 in range(0, HH, 4):
            pt = pp.tile([Cm, 512], f32, tag="fold")
            for t in range(9):
                dy, dx = t // 3 - 1, t % 3 - 1
                nc.tensor.matmul(
                    out=pt[:, :], lhsT=twf[:, t, :],
                    rhs=th[:, r0 + 1 + dy:r0 + 5 + dy, 1 + dx:W + 1 + dx],
                    start=(t == 0), stop=(t == 8))
            nc.scalar.activation(out=txc[:, r0:r0 + 4, :], in_=pt[:, :],
                                 func=AF.Silu, bias=tbc[:, :], scale=1.0)
            ptz = pp.tile([Cm, 512], f32, tag="z")
            nc.tensor.matmul(out=ptz[:, :], lhsT=twz[:, :],
                             rhs=th[:, r0 + 1:r0 + 5, 1:W + 1],
                             start=True, stop=True)
            nc.scalar.activation(out=tz[:, r0:r0 + 4, :], in_=ptz[:, :],
                                 func=AF.Silu, bias=0.0, scale=1.0)
            ptb = pp8.tile([8, 512], f32, tag="bc")
            nc.tensor.matmul(out=ptb[:, :], lhsT=twbc[:, :],
                             rhs=txc[:, r0:r0 + 4, :], start=True, stop=True)
            nc.vector.tensor_copy(out=tbcr[:, r0 * W:(r0 + 4) * W], in_=ptb[:, :])

            if r0 % 16 == 12:  # flush every 16 rows
                rs = r0 - 12
                nc.sync.dma_start(out=xc_out[:, rs * W:(r0 + 4) * W],
                                  in_=txc[:, rs:r0 + 4, :])
                nc.sync.dma_start(out=z_out[:, rs * W:(r0 + 4) * W],
                                  in_=tz[:, rs:r0 + 4, :])
        nc.sync.dma_start(out=bcr_out[:, :], in_=tbcr[:, :])
    return nc


# ------------------------------------------------------------------- L2
# tensor_tensor_scan only runs on DVE (compiler rejects it on Pool), and
# Pool tensor_tensor computes garbage on HW, so all elementwise stays on
# DVE. Reversed-AP scans cost ~2x, so the host pre-reverses the reverse
# direction's arrays and both directions scan forward here.


def build_l2():
    nc = _new_nc()
    u_f = nc.dram_tensor("u_f", [Cm, LP], bf16, kind="ExternalInput")
    u_r = nc.dram_tensor("u_r", [Cm, LP], bf16, kind="ExternalInput")
    ub_f = nc.dram_tensor("ub_f", [Cm, LP], bf16, kind="ExternalInput")
    ub_r = nc.dram_tensor("ub_r", [Cm, LP], bf16, kind="ExternalInput")
    wdt_f = nc.dram_tensor("wdt_f", [Cm, Cm], bf16, kind="ExternalInput")
    wdt_r = nc.dram_tensor("wdt_r", [Cm, Cm], bf16, kind="ExternalInput")
    dtb_f = nc.dram_tensor("dtb_f", [Cm, 1], f32, kind="ExternalInput")
    dtb_r = nc.dram_tensor("dtb_r", [Cm, 1], f32, kind="ExternalInput")
    a_f = nc.dram_tensor("a_f", [Cm, 1], f32, kind="ExternalInput")
    a_r = nc.dram_tensor("a_r", [Cm, 1], f32, kind="ExternalInput")
    hf_out = nc.dram_tensor("hf_out", [Cm, L], bf16, kind="ExternalOutput")
    hr_out = nc.dram_tensor("hr_out", [Cm, L], bf16, kind="ExternalOutput")

    EXT = CH + WU  # 2112
    with tile.TileContext(nc) as tc, \
         tc.tile_pool(name="w", bufs=1) as wp, \
         tc.tile_pool(name="u", bufs=1) as up, \
         tc.tile_pool(name="e1", bufs=2) as e1p, \
         tc.tile_pool(name="av", bufs=2) as avp, \
         tc.tile_pool(name="bt", bufs=2) as btp, \
         tc.tile_pool(name="h", bufs=3) as hp, \
         tc.tile_pool(name="psa", bufs=2, space="PSUM") as psa, \
         tc.tile_pool(name="psb", bufs=1, space="PSUM") as psb:
        twf_ = wp.tile([Cm, Cm], bf16)
        twr_ = wp.tile([Cm, Cm], bf16)
        nc.sync.dma_start(out=twf_, in_=wdt_f[:, :])
        nc.sync.dma_start(out=twr_, in_=wdt_r[:, :])
        tbf = wp.tile([Cm, 1], f32)
        tbr = wp.tile([Cm, 1], f32)
        taf = wp.tile([Cm, 1], f32)
        tar = wp.tile([Cm, 1], f32)
        nc.sync.dma_start(out=tbf, in_=dtb_f[:, :])
        nc.sync.dma_start(out=tbr, in_=dtb_r[:, :])
        nc.sync.dma_start(out=taf, in_=a_f[:, :])
        nc.sync.dma_start(out=tar, in_=a_r[:, :])

        twarm = wp.tile([Cm, 1], bf16)
        nc.vector.memset(twarm, 0.0)
        nc.scalar.activation(out=twarm, in_=twarm, func=AF.Exp,
                             bias=0.0, scale=1.0)

        tuf = up.tile([Cm, LP], bf16)
        tur = up.tile([Cm, LP], bf16)
        tubf = up.tile([Cm, LP], bf16)
        tubr = up.tile([Cm, LP], bf16)
        # first slice = exactly chunk 0's span so compute starts ASAP;
        # fwd arrays first (chunk f0 is issued first)
        SLR = (LP - EXT) // 6  # 2400
        bounds = [0, EXT] + [EXT + SLR * (i + 1) for i in range(6)]
        arrs = [(tuf, u_f), (tubf, ub_f), (tur, u_r), (tubr, ub_r)]
        for t, d in arrs:
            s = slice(0, EXT)
            nc.sync.dma_start(out=t[:, s], in_=d[:, s])
        for i in range(1, 7):
            s = slice(bounds[i], bounds[i + 1])
            for t, d in arrs:
                nc.sync.dma_start(out=t[:, s], in_=d[:, s])

        dirs = {
            'f': (twf_, tbf, taf, tuf, tubf, hf_out),
            'r': (twr_, tbr, tar, tur, tubr, hr_out),
        }
        PIECES = [(0, 1024), (1024, 1088)]

        def chunk(d, ci):
            tw, tb, ta, tu, tub, hout = dirs[d]
            sp0 = ci * CH  # padded-coords span start, len EXT
            # softplus via exp->ln (both in the natural_log_exp ACT table)
            te1 = e1p.tile([Cm, EXT], bf16, tag="e1" + d)
            for po, pl in PIECES:
                pool = psa if po == 0 else psb
                pt = pool.tile([Cm, pl], f32, tag="dtd%d" % po)
                for so in range(0, pl, 512):
                    sl_len = min(512, pl - so)
                    nc.tensor.matmul(
                        out=pt[:, so:so + sl_len], lhsT=tw[:, :],
                        rhs=tu[:, sp0 + po + so: sp0 + po + so + sl_len],
                        start=True, stop=True)
                nc.scalar.activation(out=te1[:, po:po + pl], in_=pt[:, 0:pl],
                                     func=AF.Exp, bias=tb[:, :], scale=1.0)
            # dt = ln(1 + e1) in place: te1 becomes dt
            nc.scalar.activation(out=te1, in_=te1, func=AF.Ln,
                                 bias=1.0, scale=1.0)
            tdt = te1
            tav = avp.tile([Cm, EXT], bf16, tag="av" + d)
            nc.scalar.activation(out=tav, in_=tdt, func=AF.Exp,
                                 bias=0.0, scale=ta[:, :])
            tbt = btp.tile([Cm, EXT], bf16, tag="bt" + d)
            nc.vector.tensor_tensor(out=tbt, in0=tdt,
                                    in1=tub[:, sp0:sp0 + EXT], op=MULT)
            th = hp.tile([Cm, EXT], bf16, tag="h" + d)
            nc.vector.tensor_tensor_scan(out=th, data0=tav, data1=tbt,
                                         initial=0.0, op0=MULT, op1=ADD)
            # h-out triggers wait on the scan; keep them OFF the ACT queue
            # (an ACT-issued DMA wait would head-block later ln/exp ops).
            nc.sync.dma_start(out=hout[:, ci * CH:(ci + 1) * CH], in_=th[:, WU:EXT])

        for ci in range(NCH):
            chunk('f', ci)
            chunk('r', ci)
    return nc


# ------------------------------------------------------------------- L3
def build_l3(with_lnb):
    nc = _new_nc()
    y_in = nc.dram_tensor("y_in", [Cm, LH], bf16, kind="ExternalInput")
    z_in = nc.dram_tensor("z_in", [Cm, LH], bf16, kind="ExternalInput")
    wcen = nc.dram_tensor("wcen", [Cm, Cm], bf16, kind="ExternalInput")   # I - J/128
    wvar = nc.dram_tensor("wvar", [Cm, Cm], bf16, kind="ExternalInput")   # J/128
    wfin = nc.dram_tensor("wfin", [Cm, C2], bf16, kind="ExternalInput")   # lhsT
    wfinb = nc.dram_tensor("wfinb", [Cm, C2], bf16, kind="ExternalInput") # lhsT (ln_b path)
    d_out = nc.dram_tensor("d_out", [C2, LH], bf16, kind="ExternalOutput")

    NC3 = LH // 512  # 16
    with tile.TileContext(nc) as tc, \
         tc.tile_pool(name="w", bufs=1) as wp, \
         tc.tile_pool(name="d", bufs=1) as dp, \
         tc.tile_pool(name="c", bufs=4) as cp, \
         tc.tile_pool(name="pc", bufs=3, space="PSUM") as ppc, \
         tc.tile_pool(name="pv", bufs=2, space="PSUM") as ppv, \
         tc.tile_pool(name="pd", bufs=3, space="PSUM") as ppd:
        twc = wp.tile([Cm, Cm], bf16)
        nc.sync.dma_start(out=twc, in_=wcen[:, :])
        twv = wp.tile([Cm, Cm], bf16)
        nc.sync.dma_start(out=twv, in_=wvar[:, :])
        twa = wp.tile([Cm, 128], bf16)
        twb = wp.tile([Cm, 128], bf16)
        nc.scalar.dma_start(out=twa, in_=wfin[:, 0:128])
        nc.scalar.dma_start(out=twb, in_=wfin[:, 128:256])
        if with_lnb:
            twba = wp.tile([Cm, 128], bf16)
            twbb = wp.tile([Cm, 128], bf16)
            nc.scalar.dma_start(out=twba, in_=wfinb[:, 0:128])
            nc.scalar.dma_start(out=twbb, in_=wfinb[:, 128:256])

        teps = wp.tile([Cm, 1], f32)
        nc.vector.memset(teps, 1e-5)
        twarm = wp.tile([Cm, 1], bf16)
        nc.vector.memset(twarm, 1.0)
        nc.scalar.activation(out=twarm, in_=twarm, func=AF.Ln,
                             bias=0.0, scale=1.0)

        ty = dp.tile([Cm, LH], bf16)
        tz = dp.tile([Cm, LH], bf16)
        l3b = [0, 512, 2048 + 512] + [2048 * i + 512 for i in range(2, 4)] + [LH]
        for i in range(len(l3b) - 1):
            s = slice(l3b[i], l3b[i + 1])
            nc.sync.dma_start(out=ty[:, s], in_=y_in[:, s])
            nc.sync.dma_start(out=tz[:, s], in_=z_in[:, s])

        # software-pipelined issue: stagger stages across chunks so each
        # in-order engine queue (esp. ACT) never head-blocks on a fresh dep.
        pcs, pvs, tts = {}, {}, {}

        def stage0(k):
            s = slice(k * 512, (k + 1) * 512)
            pc = ppc.tile([Cm, 512], f32, tag="c")
            pcs[k] = pc
            nc.tensor.matmul(out=pc, lhsT=twc[:, :], rhs=ty[:, s],
                             start=True, stop=True)
            tsq = cp.tile([Cm, 512], bf16, tag="sq")
            nc.scalar.activation(out=tsq, in_=pc, func=AF.Square,
                                 bias=0.0, scale=1.0)
            pv = ppv.tile([Cm, 512], f32, tag="v")
            pvs[k] = pv
            nc.tensor.matmul(out=pv, lhsT=twv[:, :], rhs=tsq,
                             start=True, stop=True)

        def stage1(k):
            s = slice(k * 512, (k + 1) * 512)
            tln = cp.tile([Cm, 512], f32, tag="ln")
            nc.scalar.activation(out=tln, in_=pvs.pop(k), func=AF.Ln,
                                 bias=teps[:, :], scale=1.0)
            trs = cp.tile([Cm, 512], bf16, tag="rs")
            nc.scalar.activation(out=trs, in_=tln, func=AF.Exp,
                                 bias=0.0, scale=-0.5)
            trz = cp.tile([Cm, 512], bf16, tag="rz")
            nc.vector.tensor_tensor(out=trz, in0=trs, in1=tz[:, s], op=MULT)
            tt = cp.tile([Cm, 512], bf16, tag="t")
            tts[k] = tt
            nc.vector.tensor_tensor(out=tt, in0=pcs.pop(k), in1=trz, op=MULT)

        obuf = {}

        def stage2(k):
            s = slice(k * 512, (k + 1) * 512)
            tt = tts.pop(k)
            pda = ppd.tile([Cm, 512], f32, tag="d")
            nc.tensor.matmul(out=pda, lhsT=twa[:, :], rhs=tt,
                             start=True, stop=not with_lnb)
            if with_lnb:
                nc.tensor.matmul(out=pda, lhsT=twba[:, :], rhs=tz[:, s],
                                 start=False, stop=True)
            pdb = ppd.tile([Cm, 512], f32, tag="d")
            nc.tensor.matmul(out=pdb, lhsT=twb[:, :], rhs=tt,
                             start=True, stop=not with_lnb)
            if with_lnb:
                nc.tensor.matmul(out=pdb, lhsT=twbb[:, :], rhs=tz[:, s],
                                 start=False, stop=True)
            # batch output DMAs in chunk pairs to halve sync-queue triggers
            if k % 2 == 0:
                toa = cp.tile([128, 1024], bf16, tag="oa")
                tob = cp.tile([128, 1024], bf16, tag="ob")
                obuf['a'], obuf['b'] = toa, tob
            off = (k % 2) * 512
            nc.vector.tensor_copy(out=obuf['a'][:, off:off + 512], in_=pda)
            nc.vector.tensor_copy(out=obuf['b'][:, off:off + 512], in_=pdb)
            if k % 2 == 1:
                sp = slice((k - 1) * 512, (k + 1) * 512)
                nc.sync.dma_start(out=d_out[0:128, sp], in_=obuf['a'])
                nc.sync.dma_start(out=d_out[128:256, sp], in_=obuf['b'])

        for k in range(NC3 + 2):
            if k < NC3:
                stage0(k)
            if 1 <= k:
                if k - 1 < NC3:
                    stage1(k - 1)
            if k >= 2:
                stage2(k - 2)
    return nc


# ------------------------------------------------------------------- host
def _get_ncs(with_lnb):
    key = ("ncs", with_lnb)
    if key not in _CACHE:
        nc1, nc2, nc3 = build_l1(), build_l2(), build_l3(with_lnb)
        for n in (nc1, nc2, nc3):
            _split_multiwaits(n)
        _CACHE[key] = (nc1, nc2, nc3)
    return _CACHE[key]


def kernel(x, cv1_w, cv1_b, scale_w, in_proj_w, conv_w, conv_b, x_proj_w,
           dt_w, dt_b, A_logs, Ds, ln_g, ln_b, out_proj_w, cv2_w, cv2_b):
    f = np.float32
    x = np.asarray(x, f)
    cv1_w = np.asarray(cv1_w, f); cv1_b = np.asarray(cv1_b, f)
    in_proj_w = np.asarray(in_proj_w, f)
    conv_w = np.asarray(conv_w, f); conv_b = np.asarray(conv_b, f)
    x_proj_w = np.asarray(x_proj_w, f)
    dt_w = np.asarray(dt_w, f); dt_b = np.asarray(dt_b, f)
    A_logs = np.asarray(A_logs, f); Ds = np.asarray(Ds, f)
    ln_g = np.asarray(ln_g, f); ln_b = np.asarray(ln_b, f)
    out_proj_w = np.asarray(out_proj_w, f)
    cv2_w = np.asarray(cv2_w, f); cv2_b = np.asarray(cv2_b, f)
    scale_v = np.asarray(scale_w, f).reshape(Cm)

    Wip_x, Wip_z = in_proj_w[:Cm], in_proj_w[Cm:]
    dwk = conv_w[:, 0]
    A = -np.exp(A_logs).reshape(K, Cm)
    Dk = Ds.reshape(K, Cm)
    Dall = Dk.sum(axis=0)
    W_dtk = np.einsum('kdr,krc->kdc', dt_w, x_proj_w[:, :R])
    WB, WC = x_proj_w[:, R], x_proj_w[:, R + 1]
    W_final = cv2_w @ (scale_v[:, None] * out_proj_w)
    W1 = W_final * ln_g[None, :]
    with_lnb = bool(np.any(ln_b != 0.0))
    Wb1 = W_final * ln_b[None, :]

    # fold lhsT: (tap, k=h-chan, m=out-chan) -> host layout (k, tap, m)
    Wfold = np.einsum('cyx,cd->yxdc', dwk, Wip_x)      # (3,3, in, out)
    wfold_rm = np.ascontiguousarray(
        Wfold.reshape(9, Cm, Cm).transpose(1, 0, 2))   # row-major cores
    wbc_l = np.stack([WB[0], WC[0], WB[2], WC[2],
                      WB[1], WC[1], WB[3], WC[3]], axis=1)

    nc1, nc2, nc3 = _get_ncs(with_lnb)

    # ---------------- L1 ----------------
    l1_maps = []
    wcv1_h = np.ascontiguousarray(cv1_w.T).astype(NBF)
    wfold_h = wfold_rm.astype(NBF)
    wz_h = np.ascontiguousarray(Wip_z.T).astype(NBF)
    wbc_h = np.ascontiguousarray(wbc_l).astype(NBF)
    for core in range(8):
        b, half = core // 2, core % 2
        r0 = half * HH
        xs = np.zeros((C1, HH + 2, W), np.float32)
        lo, hi = r0 - 1, r0 + HH + 1
        slo, shi = max(lo, 0), min(hi, H)
        xs[:, slo - lo: shi - lo, :] = x[b, :, slo:shi, :]
        mask = np.ones((Cm, 2), np.float32)
        mask[:, 0] = 0.0 if half == 0 else 1.0
        mask[:, 1] = 1.0 if half == 0 else 0.0
        l1_maps.append({
            "x_in": xs.astype(NBF),
            "wcv1": wcv1_h,
            "bcv1": cv1_b.reshape(Cm, 1),
            "wfold": wfold_h,
            "bconv": conv_b.reshape(Cm, 1),
            "wz": wz_h,
            "wbc": wbc_h,
            "hmask": mask,
        })
    r1 = _run(nc1, l1_maps, "L1")

    xc = np.zeros((B, Cm, L), NBF)
    zf = np.zeros((B, Cm, L), NBF)
    rows = np.zeros((B, 8, L), f)
    for core in range(8):
        b, half = core // 2, core % 2
        sl = slice(half * LH, (half + 1) * LH)
        xc[b][:, sl] = r1[core]["xc_out"]
        zf[b][:, sl] = r1[core]["z_out"]
        rows[b][:, sl] = r1[core]["bcr_out"].astype(f)

    # ---------------- L2 ----------------
    def t_spatial(a):
        return np.ascontiguousarray(
            a.reshape(*a.shape[:-1], H, W).swapaxes(-1, -2).reshape(*a.shape[:-1], L))

    l2_maps = []
    crows = []  # per core (Cf, Cr) for host merge
    for core in range(8):
        b, g = core // 2, core % 2
        if g == 0:
            u = xc[b].astype(f)
            kf, kr = 0, 2
            br_f, cr_f = rows[b][0], rows[b][1]
            br_r, cr_r = rows[b][2], rows[b][3]
        else:
            u = t_spatial(xc[b]).astype(f)
            kf, kr = 1, 3
            br_f, cr_f = t_spatial(rows[b][4]), t_spatial(rows[b][5])
            br_r, cr_r = t_spatial(rows[b][6]), t_spatial(rows[b][7])
        upf = np.zeros((Cm, LP), f)
        upf[:, WU:WU + L] = u
        upr = np.zeros((Cm, LP), f)
        upr[:, WU:WU + L] = u[:, ::-1]
        ubf = np.zeros((Cm, LP), f)
        ubf[:, WU:WU + L] = u * br_f[None, :]
        ubr = np.zeros((Cm, LP), f)
        ubr[:, WU:WU + L] = (u * br_r[None, :])[:, ::-1]
        crows.append((cr_f, cr_r))
        l2_maps.append({
            "u_f": upf.astype(NBF),
            "u_r": upr.astype(NBF),
            "ub_f": ubf.astype(NBF),
            "ub_r": ubr.astype(NBF),
            "wdt_f": np.ascontiguousarray(W_dtk[kf].T).astype(NBF),
            "wdt_r": np.ascontiguousarray(W_dtk[kr].T).astype(NBF),
            "dtb_f": dt_b[kf].reshape(Cm, 1), "dtb_r": dt_b[kr].reshape(Cm, 1),
            "a_f": A[kf].reshape(Cm, 1).astype(f), "a_r": A[kr].reshape(Cm, 1).astype(f),
        })
    r2 = _run(nc2, l2_maps, "L2")

    # host merge: m_g = hf*Cf + hr*Cr ; y = m_0 + t(m_1) + Dall*xc
    l3_maps = []
    wcen_h = (np.eye(Cm, dtype=f) - np.full((Cm, Cm), 1.0 / Cm, f)).astype(NBF)
    wvar_h = np.full((Cm, Cm), 1.0 / Cm, f).astype(NBF)
    wfin_h = np.ascontiguousarray(W1.T).astype(NBF)
    wfinb_h = np.ascontiguousarray(Wb1.T).astype(NBF)
    for b in range(B):
        cf0, cr0 = crows[2 * b]
        cf1, cr1 = crows[2 * b + 1]
        m0 = (r2[2 * b]["hf_out"].astype(f) * cf0[None, :]
              + r2[2 * b]["hr_out"][:, ::-1].astype(f) * cr0[None, :])
        m1 = (r2[2 * b + 1]["hf_out"].astype(f) * cf1[None, :]
              + r2[2 * b + 1]["hr_out"][:, ::-1].astype(f) * cr1[None, :])
        ypre = (m0 + t_spatial(m1) + Dall[:, None] * xc[b].astype(f)).astype(NBF)
        for half in range(2):
            sl = slice(half * LH, (half + 1) * LH)
            l3_maps.append({
                "y_in": np.ascontiguousarray(ypre[:, sl]),
                "z_in": np.ascontiguousarray(zf[b][:, sl]),
                "wcen": wcen_h,
                "wvar": wvar_h,
                "wfin": wfin_h,
                "wfinb": wfinb_h,
            })
    r3 = _run(nc3, l3_maps, "L3")

    out = np.empty((B, C2, H, W), np.float32)
    for core in range(8):
        b, half = core // 2, core % 2
        sl = slice(half * LH, (half + 1) * LH)
        out[b].reshape(C2, L)[:, sl] = r3[core]["d_out"].astype(f)
    out += x
    out += cv2_b[None, :, None, None]
    return out
